# revision 24
# baseline (speedup 1.0000x reference)
"""Multi-head dilated sliding-window attention (window=129, dil=1) on 8 TRN2 cores.

Sharding: sequence-parallel. Each core computes 256 query rows (N=2048 / 8),
with a 64-row K/V halo on each side (zero-padded at the sequence edges).
Weights are replicated (streamed to SBUF, bf16).

Band-softmax identity (reference softmaxes the FULL row with zeros outside
the band):
    out_i = (sum_band (e^{s_ij} - 1) V_j + sum_all V_j) / (sum_band (e^{s_ij} - 1) + N)
with V_raw = x@Wv (bv folded into bo2 = bv@Wo + bo on host), bk added as a
per-partition scalar on the K^T copy (padding keys are excluded by per-core
edge-masked band masks), and sum_all V_j = (sum_n x_n)@Wv computed on host.

Host-side layout prep: x arrives pre-transposed (xT[e,seq]), Wq/Wk arrive
column-block-major so round db's weight slice is one contiguous DMA.

Schedule: one DMA issue stream on the Sync engine in consumption order
(issue cost ~650ns each serializes on Sync; transfers fan out over all 16
DMA engines at ~344 GB/s aggregate). V = x@Wv streams et-major right behind
the DMA. Then 8 rounds, one head-pair each: Q^T/K^T projections, previous
round's PV + epilogue, scores + exp chain (exp on ACT, -1 on GpSimd, mask
mul on DVE). Output projection runs as one dense block at the end from the
accumulated A^T tiles.
"""

import numpy as np
import ml_dtypes
from contextlib import ExitStack

import concourse.bass as bass
import concourse.tile as tile
from concourse import bacc, mybir
from concourse.bass_utils import run_bass_kernel_spmd

F32 = mybir.dt.float32
BF16 = mybir.dt.bfloat16
NPBF16 = ml_dtypes.bfloat16
N, E, H, D = 2048, 1024, 16, 64
R = N // 8          # 256 query rows per core
HALO = R + 128      # 384 K/V rows per core
NQB = R // 128      # query blocks per core


def build_graph():
    nc = bacc.Bacc("TRN2", target_bir_lowering=False, debug=False, num_devices=8)

    xT_d = nc.declare_dram_parameter("xT", [128, 8 * HALO], BF16, isOutput=False)
    wqk_d = nc.declare_dram_parameter("wqk", [E, 2048], BF16, isOutput=False)
    wv_d = nc.declare_dram_parameter("Wv", [E, H * D], BF16, isOutput=False)
    wo_d = nc.declare_dram_parameter("Wo", [H * D, E], BF16, isOutput=False)
    sm_d = nc.declare_dram_parameter("smalls", [128, 16], F32, isOutput=False)
    mb_d = nc.declare_dram_parameter("maskb", [128, 640], BF16, isOutput=False)
    bc_d = nc.declare_dram_parameter("biascat", [1, H * (D + 1)], BF16,
                                     isOutput=False)
    bo2_d = nc.declare_dram_parameter("bo2t", [128, E], BF16, isOutput=False)
    out_d = nc.declare_dram_parameter("out", [R, E], BF16, isOutput=True)

    with tile.TileContext(nc) as tc, ExitStack() as ctx:
        const = ctx.enter_context(tc.tile_pool(name="const", bufs=1))
        pers = ctx.enter_context(tc.tile_pool(name="pers", bufs=1))
        qtp = ctx.enter_context(tc.tile_pool(name="qtp", bufs=2))
        ktp = ctx.enter_context(tc.tile_pool(name="ktp", bufs=2))
        epool = ctx.enter_context(tc.tile_pool(name="epool", bufs=3))
        ppool = ctx.enter_context(tc.tile_pool(name="ppool", bufs=4))
        apool = ctx.enter_context(tc.tile_pool(name="apool", bufs=3))
        zpool = ctx.enter_context(tc.tile_pool(name="zpool", bufs=8))
        obpool = ctx.enter_context(tc.tile_pool(name="obpool", bufs=2))
        psum = ctx.enter_context(tc.tile_pool(name="psum", bufs=6, space="PSUM"))
        psum2 = ctx.enter_context(tc.tile_pool(name="psum2", bufs=2, space="PSUM"))

        def ps(shape, dt=F32):
            return psum.tile(shape, dt, tag="ps", name="pst")

        # ---- SBUF tiles -------------------------------------------------
        xT = const.tile([128, 8, HALO], BF16, tag="xT")
        wqk_t = [const.tile([128, 2, 8, 128], BF16, tag=f"wqk{db}", name="wqkt")
                 for db in range(8)]
        wv_t = [const.tile([128, E], BF16, tag=f"wv{et}", name="wvt") for et in range(8)]
        wo_t = [const.tile([128, E], BF16, tag=f"wo{db}", name="wot") for db in range(8)]
        smalls = const.tile([128, 16], F32, tag="smalls")
        bq_sb = smalls[:, 0:8]
        bk_sb = smalls[:, 8:16]
        maskb = const.tile([128, 640], BF16, tag="maskb")
        m4 = maskb[:, 0:512]
        ident = maskb[:, 512:640]
        biascat = const.tile([1, H, D + 1], BF16, tag="biascat")
        bo2 = const.tile([128, E], BF16, tag="bo2")
        ones_sb = const.tile([1, 128], BF16, tag="ones")
        nc.vector.memset(ones_sb[:], 1.0)

        Vaug = pers.tile([128, 3, H, D + 1], BF16, tag="Vaug")
        AT = pers.tile([128, 8, R], BF16, tag="AT")  # [d_p, db, q]

        # ---- single DMA issue stream on Sync, consumption order ---------
        # startup-critical transfers in fine chunks: V's et-step k needs
        # xT[et=k] and wv_t[k]; interleave so et0/et1 data lands earliest
        nc.sync.dma_start(xT[:, 0:1, :], xT_d[:, 0:HALO])
        for hf in range(2):
            nc.sync.dma_start(wv_t[0][:, hf * 512:(hf + 1) * 512],
                              wv_d[0:128, hf * 512:(hf + 1) * 512])
        nc.sync.dma_start(xT[:, 1:2, :], xT_d[:, HALO:2 * HALO])
        for hf in range(2):
            nc.sync.dma_start(wv_t[1][:, hf * 512:(hf + 1) * 512],
                              wv_d[128:256, hf * 512:(hf + 1) * 512])
        nc.sync.dma_start(xT[:, 2:3, :], xT_d[:, 2 * HALO:3 * HALO])
        nc.sync.dma_start(wv_t[2][:], wv_d[256:384, :])
        nc.sync.dma_start(xT[:, 3:4, :], xT_d[:, 3 * HALO:4 * HALO])
        nc.sync.dma_start(wv_t[3][:], wv_d[384:512, :])
        nc.sync.dma_start(xT[:, 4:6, :], xT_d[:, 4 * HALO:6 * HALO])
        nc.sync.dma_start(wv_t[4][:], wv_d[512:640, :])
        nc.sync.dma_start(xT[:, 6:8, :], xT_d[:, 6 * HALO:8 * HALO])
        for et in range(5, 8):
            nc.sync.dma_start(wv_t[et][:], wv_d[et * 128:(et + 1) * 128, :])
        # round-0 weights + small consts jump the sync queue via the scalar
        # engine's hardware DGE (scalar is idle until the first exp)
        nc.scalar.dma_start(wqk_t[0][:, 0, :, :], wqk_d[0:128, 0:1024])
        nc.scalar.dma_start(wqk_t[0][:, 1, :, :], wqk_d[0:128, 1024:2048])
        nc.scalar.dma_start(smalls[:], sm_d[:, :])
        nc.scalar.dma_start(maskb[:], mb_d[:, :])
        nc.scalar.dma_start(biascat[:], bc_d[:, :])
        nc.scalar.dma_start(wo_t[7][:], wo_d[7 * 128:8 * 128, :])
        nc.scalar.dma_start(bo2[:], bo2_d[:, :])
        for db in range(1, 6):
            nc.sync.dma_start(wqk_t[db][:], wqk_d[db * 128:(db + 1) * 128, :])
            nc.sync.dma_start(wo_t[db - 1][:],
                              wo_d[(db - 1) * 128:db * 128, :])
        nc.sync.dma_start(wqk_t[6][:], wqk_d[6 * 128:7 * 128, :])
        nc.sync.dma_start(wqk_t[7][:], wqk_d[7 * 128:8 * 128, :])
        for db in range(5, 7):
            nc.sync.dma_start(wo_t[db][:], wo_d[db * 128:(db + 1) * 128, :])

        # ---- PE clock warm-up during the x/wv DMA wait ------------------
        wu = const.tile([128, 128], BF16, tag="wu")
        nc.vector.memset(wu[:], 0.0)
        wups = psum.tile([128, 128], F32, tag="ps", name="wups")
        for _ in range(52):
            nc.tensor.matmul(wups[:], wu[:], wu[:], start=True, stop=True)

        # ---- V projection pass A (heads 0-7): streams behind the DMA;
        # pass B (heads 8-15) is interleaved st-serially into rounds 0-2
        vps = [psum.tile([128, 512], F32, tag="ps", name="vps") for _ in range(3)]
        for et in range(8):
            for st in range(3):
                nc.tensor.matmul(vps[st][:],
                                 xT[:, et, st * 128:(st + 1) * 128],
                                 wv_t[et][:, 0:512],
                                 start=(et == 0), stop=(et == 7))
        for st in range(3):
            src = vps[st][:].rearrange("p (h d) -> p h d", d=D)
            dst = Vaug[:, st, 0:8, 0:D]
            if st == 0:
                nc.scalar.copy(dst, src)
            else:
                nc.vector.tensor_copy(dst, src)
        nc.vector.memset(Vaug[:, :, 0:8, D:D + 1], 1.0)

        def v_pass_b(st):
            vpb = ps([128, 512])
            for et in range(8):
                nc.tensor.matmul(vpb[:], xT[:, et, st * 128:(st + 1) * 128],
                                 wv_t[et][:, 512:1024],
                                 start=(et == 0), stop=(et == 7))
            src = vpb[:].rearrange("p (h d) -> p h d", d=D)
            if st == 0:
                nc.scalar.copy(Vaug[:, st, 8:16, 0:D], src)
            else:
                nc.vector.tensor_copy(Vaug[:, st, 8:16, 0:D], src)
            if st == 2:
                nc.vector.memset(Vaug[:, :, 8:16, D:D + 1], 1.0)

        # ---- fused rounds: one head-pair db per round --------------------
        def pv_mms(pr):
            db, ptl = pr
            pv = ps([128, 2, 2, 65])  # [qblk, i, D+1] in one bank
            first = True
            for qblk in range(NQB):
                for i in range(2):
                    for cblk in range(2):
                        quad = qblk * 2 + cblk
                        nc.tensor.matmul(pv[:, qblk, i, :],
                                         ptl[i][:, quad * 128:(quad + 1) * 128],
                                         Vaug[:, qblk + cblk, 2 * db + i, :],
                                         start=first, stop=False)
                        first = False
            for qblk in range(NQB):
                for i in range(2):
                    nc.tensor.matmul(pv[:, qblk, i, :], ones_sb[0:1, :],
                                     biascat[0:1, 2 * db + i, :], start=False,
                                     stop=(qblk == 1 and i == 1))
            return pv

        def pv_epilogue(db, pv):
            asc = apool.tile([128, 2, 128], BF16, tag="asc", name="asc")
            zin = zpool.tile([128, 2, 2], F32, tag="z", name="zin")
            nc.vector.reciprocal(zin[:], pv[:, :, :, 64:65])
            for qblk in range(NQB):
                for i in range(2):
                    nc.vector.tensor_scalar_mul(
                        asc[:, qblk, i * 64:(i + 1) * 64],
                        pv[:, qblk, i, 0:64], zin[:, qblk, i:i + 1])
            return asc

        def at_transposes(db, asc):
            tp = ps([128, 2, 128], BF16)
            for qblk in range(NQB):
                nc.tensor.transpose(tp[:, qblk, :], asc[:, qblk, :], ident[:])
            for qblk in range(NQB):
                nc.scalar.copy(AT[:, db, qblk * 128:(qblk + 1) * 128],
                               tp[:, qblk, :])

        def out_mms_hf0(db, opsl):
            for qblk in range(NQB):
                nc.tensor.matmul(opsl[qblk][:],
                                 AT[:, db, qblk * 128:(qblk + 1) * 128],
                                 wo_t[db][:, 0:512],
                                 start=(db == 0), stop=(db == 7))

        prev = None  # (db, ptiles)
        asc_hist = {}
        ops0 = [psum2.tile([128, 512], F32, tag="ops0", name="ops0")
                for _ in range(NQB)]
        for r in range(10):
            if r < 8:
                db = r
                qp = ps([128, R])
                for et in range(8):
                    nc.tensor.matmul(qp[:], wqk_t[db][:, 0, et, :],
                                     xT[:, et, 64:64 + R],
                                     start=(et == 0), stop=(et == 7))
                qt = qtp.tile([128, R], BF16, tag="qt", name="qt")
                nc.scalar.add(qt[:], qp[:], bq_sb[:, db:db + 1])
                kp = ps([128, HALO])
                for et in range(8):
                    nc.tensor.matmul(kp[:], wqk_t[db][:, 1, et, :],
                                     xT[:, et, :],
                                     start=(et == 0), stop=(et == 7))
                kt = ktp.tile([128, HALO], BF16, tag="kt", name="kt")
                nc.scalar.add(kt[:], kp[:], bk_sb[:, db:db + 1])
                if r >= 2:
                    at_transposes(r - 2, asc_hist.pop(r - 2))
                    out_mms_hf0(r - 2, ops0)
                if prev is not None:
                    pvp = pv_mms(prev)
                    asc_hist[prev[0]] = pv_epilogue(prev[0], pvp)
                ptl = {}
                for i in range(2):
                    rr = i * 64
                    sp = ps([128, 512])
                    nc.tensor.matmul(sp[:, 0:128], kt[rr:rr + 64, 0:128],
                                     qt[rr:rr + 64, 0:128],
                                     start=True, stop=False)
                    nc.tensor.matmul(sp[:, 128:384], kt[rr:rr + 64, 128:256],
                                     qt[rr:rr + 64, 0:256],
                                     start=False, stop=False)
                    nc.tensor.matmul(sp[:, 384:512], kt[rr:rr + 64, 256:384],
                                     qt[rr:rr + 64, 128:256],
                                     start=False, stop=True)
                    et_ = epool.tile([128, 512], BF16, tag="e", name="et_")
                    nc.scalar.activation(et_[:], sp[:],
                                         mybir.ActivationFunctionType.Exp)
                    nc.vector.tensor_scalar_add(et_[:], et_[:], -1.0)
                    pt = ppool.tile([128, 512], BF16, tag="p", name="pt")
                    nc.vector.tensor_mul(pt[:], et_[:], m4)
                    ptl[i] = pt
                if r < 3:
                    v_pass_b(r)
                prev = (db, ptl)
            elif r == 8:
                pvp = pv_mms(prev)
                asc_hist[prev[0]] = pv_epilogue(prev[0], pvp)
                at_transposes(r - 2, asc_hist.pop(r - 2))
                out_mms_hf0(r - 2, ops0)

        # ---- output tail: hf1 mms for db0-5 fill the last epilogue latency
        ops1l = [psum.tile([128, 512], F32, tag="ps", name="ops1")
                 for _ in range(NQB)]
        for qblk in range(NQB):
            for db in range(6):
                nc.tensor.matmul(ops1l[qblk][:],
                                 AT[:, db, qblk * 128:(qblk + 1) * 128],
                                 wo_t[db][:, 512:1024],
                                 start=(db == 0), stop=False)
        at_transposes(7, asc_hist.pop(7))
        out_mms_hf0(7, ops0)
        for qblk in range(NQB):
            for db in range(6, 8):
                nc.tensor.matmul(ops1l[qblk][:],
                                 AT[:, db, qblk * 128:(qblk + 1) * 128],
                                 wo_t[db][:, 512:1024],
                                 start=False, stop=(db == 7))
            ob = obpool.tile([128, E], BF16, tag="ob", name="ob")
            eng = nc.sync if qblk == 0 else nc.scalar
            nc.vector.tensor_tensor(ob[:, 0:512], ops0[qblk][:], bo2[:, 0:512],
                                    mybir.AluOpType.add)
            eng.dma_start(out_d[qblk * 128:(qblk + 1) * 128, 0:512],
                          ob[:, 0:512])
            nc.vector.tensor_tensor(ob[:, 512:1024], ops1l[qblk][:],
                                    bo2[:, 512:1024], mybir.AluOpType.add)
            eng.dma_start(out_d[qblk * 128:(qblk + 1) * 128, 512:1024],
                          ob[:, 512:1024])

    nc.compile()
    return nc


_NC = None


def get_nc():
    global _NC
    if _NC is None:
        _NC = build_graph()
    return _NC


def make_in_maps(x, Wq, bq, Wk, bk, Wv, bv, Wo, bo):
    f = lambda a: np.ascontiguousarray(np.asarray(a, dtype=np.float32))
    bf = lambda a: np.ascontiguousarray(
        np.asarray(a, dtype=np.float32).astype(NPBF16))
    x2 = f(x).reshape(N, E)
    Wqf, Wkf, Wvf, Wof = f(Wq), f(Wk), f(Wv), f(Wo)
    # Wq/Wk column-block-major: wqk[db, p, qk, et, j] = W[et*128+p, db*128+j]
    wqT = Wqf.reshape(8, 128, 8, 128).transpose(2, 1, 0, 3)
    wkT = Wkf.reshape(8, 128, 8, 128).transpose(2, 1, 0, 3)
    wqk = np.stack([wqT, wkT], axis=2).reshape(E, 2048)
    # band masks per quadrant [m0 | m1 | m0 | m1]
    ci = np.arange(128, dtype=np.float32)[:, None]  # key index c (partitions)
    qi = np.arange(128, dtype=np.float32)[None, :]  # query index q (free)
    m0 = (ci >= qi).astype(np.float32)
    m1 = (ci <= qi).astype(np.float32)
    mask4 = np.concatenate([m0, m1, m0, m1], axis=1)
    # biascat rows: [SV_h (64) | N] per head, SV = (sum_n x_n) @ Wv
    SV = x2.sum(0, dtype=np.float32) @ Wvf
    bcat = np.zeros((H, D + 1), np.float32)
    bcat[:, 0:D] = SV.reshape(H, D)
    bcat[:, D] = float(N)
    bo2 = f(bv) @ Wof + f(bo)
    common = {
        "wqk": bf(wqk),
        "Wv": bf(Wvf), "Wo": bf(Wof),
        "biascat": bcat.reshape(1, H * (D + 1)).astype(NPBF16),
        "bo2t": np.ascontiguousarray(np.tile(bo2[None, :], (128, 1)).astype(NPBF16)),
    }
    in_maps = []
    for c in range(8):
        r0 = c * R
        xh = np.zeros((HALO, E), np.float32)
        lo, hi = r0 - 64, r0 + R + 64
        slo, shi = max(lo, 0), min(hi, N)
        xh[slo - lo: shi - lo] = x2[slo:shi]
        xTh = xh.T.reshape(8, 128, HALO).transpose(1, 0, 2).reshape(128, 8 * HALO)
        m4c = mask4.copy()
        if c == 0:    # halo rows 0:64 are padding, used only by quad 0
            m4c[0:64, 0:128] = 0.0
        if c == 7:    # halo rows 320:384 are padding, used only by quad 3
            m4c[64:128, 384:512] = 0.0
        sm = np.zeros((128, 16), np.float32)
        sm[:, 0:8] = f(bq).reshape(8, 128).T
        sm[:, 8:16] = f(bk).reshape(8, 128).T
        mb = np.zeros((128, 640), np.float32)
        mb[:, 0:512] = m4c
        mb[:, 512:640] = np.eye(128, dtype=np.float32)
        in_maps.append({**common,
                        "xT": np.ascontiguousarray(xTh.astype(NPBF16)),
                        "smalls": np.ascontiguousarray(sm),
                        "maskb": np.ascontiguousarray(mb.astype(NPBF16))})
    return in_maps


def kernel(x, Wq, bq, Wk, bk, Wv, bv, Wo, bo, _trace=False, _trace_kwargs=None):
    nc = get_nc()
    in_maps = make_in_maps(x, Wq, bq, Wk, bk, Wv, bv, Wo, bo)
    res = run_bass_kernel_spmd(nc, in_maps, list(range(8)), trace=_trace,
                               **(_trace_kwargs or {}))
    out = np.concatenate([np.asarray(res.results[c]["out"], dtype=np.float32)
                          for c in range(8)], axis=0)
    kernel.last_result = res
    return out[None].astype(np.float32)
